# revision 31
# baseline (speedup 1.0000x reference)
"""VQ codebook kernel for Trainium2 (8 NeuronCores, data-parallel over B).

Per core b: z = z_real[b] (4096, 256).
  scores t[i,j] = z_i . e_j - 0.5*||e_j||^2   (argmax_j t = argmin_j ||z_i - e_j||^2)
  computed on PE in fp16 (z and codebook host-cast to fp16; z^T loaded via the
  16-bit DMA xbar transpose straight from DRAM; bias as a rank-1 fp16 matmul).
  argmax via DVE InstMax + InstMaxIndex (top-8 values+indices kept).
  codebook gather via per-tile gpsimd indirect DMA (fp32 emb rows from DRAM).
  vq_loss identity: sum((zq-z)^2) = sum_i ||z_i||^2 - 2*sum_i max_i
    (||z||^2 exactly in fp32 via ACT Square+accum on device).
Host post-pass: rows whose top-2 fp16 scores are within MARGIN are rescored
exactly (fp64) among the device's top-8 candidates, fixing any fp16-induced
argmax flips; this reproduces full-fp32 fidelity (reference-vs-fp64 flips only
occur below fp32 noise, far inside MARGIN).
z_imag passes through on the host.
"""

import os
import sys

import numpy as np

sys.path.insert(0, "/opt/trn_rl_repo")

B, L, D, NE = 8, 4096, 256, 1024
P = 128  # partitions
NT = L // P  # 32 row tiles per core
NB = NE // 512  # 2 PSUM banks of 512 scores

_compiled = {}
LAST_RESULT = None

# rescore rows whose top-2 device scores are closer than the max plausible
# device score error (worst-case fp16 product rounding ~0.1 + fp16 bias
# rounding ~0.06, x2+ safety)
MARGIN = 0.5


def _build():
    import concourse.bass as bass
    import concourse.mybir as mybir
    from concourse import bacc
    from concourse.tile import TileContext

    f32 = mybir.dt.float32
    f16 = mybir.dt.float16

    nc = bacc.Bacc("TRN2", target_bir_lowering=False, debug=False)

    z16_dram = nc.dram_tensor("z16", [L, D], f16, kind="ExternalInput").ap()
    embT_dram = nc.dram_tensor("embT16", [D, NE], f16, kind="ExternalInput").ap()
    ebias_dram = nc.dram_tensor("ebias16", [1, NE], f16, kind="ExternalInput").ap()
    emb_dram = nc.dram_tensor("emb", [NE, D], f32, kind="ExternalInput").ap()
    zq_dram = nc.dram_tensor("zq", [L, D], f32, kind="ExternalOutput").ap()
    m8_dram = nc.dram_tensor("m8out", [P, NT * 8], f32, kind="ExternalOutput").ap()
    i8_dram = nc.dram_tensor(
        "i8out", [P, NT * 8], mybir.dt.uint32, kind="ExternalOutput"
    ).ap()

    with TileContext(nc) as tc:
        with (
            tc.tile_pool(name="persist", bufs=1) as persist,
            tc.tile_pool(name="sc", bufs=6) as sc_pool,
            tc.tile_pool(name="ps_s", bufs=4, space="PSUM") as ps_s,
        ):
            # --- codebook (transposed, fp16) + bias row + ones row FIRST so
            # the PE warmup and first tiles start as early as possible
            eT = []
            for k in range(2):
                t = persist.tile([P, NE], f16, tag=f"eT{k}")
                nc.sync.dma_start(out=t[:], in_=embT_dram[k * P : (k + 1) * P, :])
                eT.append(t)
            ebias = persist.tile([1, NE], f16, tag="ebias")
            nc.sync.dma_start(out=ebias[:], in_=ebias_dram[:])
            ones = persist.tile([1, P], f16, tag="ones")
            nc.vector.memset(ones[:], 1.0)

            # --- z^T via 16-bit xbar transpose: zt[k] [128, 4096] fp16
            # (interleave k so tile 0's both k-slices arrive first)
            zt0 = persist.tile([P, L], f16, tag="zt0")
            zt1 = persist.tile([P, L], f16, tag="zt1")
            zt = [zt0, zt1]
            for c in range(4):
                rs = slice(c * (L // 4), (c + 1) * (L // 4))
                for k in range(2):
                    nc.sync.dma_start(
                        out=zt[k][:, rs],
                        in_=z16_dram[rs, k * P : (k + 1) * P],
                        transpose=True,
                    )

            # --- accumulators
            s_all = persist.tile([P, NT * NE], f32, tag="s_all")
            m_all = persist.tile([P, NT * 8], f32, tag="m_all")
            idx_all = persist.tile([P, NT * 8], mybir.dt.uint32, tag="idx_all")
            zq_sb = persist.tile([P, NT * D], f32, tag="zq_sb")

            # --- PE warmup burst (engage HAM 2.4 GHz before the real work);
            # sourced from a memset tile so it needs no DMA and starts at t=0
            wsrc = persist.tile([P, 512], f16, tag="wsrc")
            nc.vector.memset(wsrc[:], 0.5)
            warm_ps = ps_s.tile([P, 512], f32, tag="s_ps")
            for _ in range(14):
                nc.tensor.matmul(
                    warm_ps[:], lhsT=wsrc[:, 0:P], rhs=wsrc[:],
                    start=True, stop=True,
                )

            # --- main loop over row tiles
            for t in range(NT):
                s_ps = ps_s.tile([P, NE], f32)
                # k-major order so consecutive matmuls share the stationary
                # operand (fewer weight reloads); bank accumulation groups
                # interleave via start/stop flags per PSUM region.
                for k in range(2):
                    for b in range(NB):
                        cs = slice(b * 512, (b + 1) * 512)
                        nc.tensor.matmul(
                            s_ps[:, cs],
                            lhsT=zt[k][:, t * P : (t + 1) * P],
                            rhs=eT[k][:, cs],
                            start=(k == 0),
                            stop=False,
                        )
                for b in range(NB):
                    cs = slice(b * 512, (b + 1) * 512)
                    nc.tensor.matmul(
                        s_ps[:, cs],
                        lhsT=ones[:, :],
                        rhs=ebias[:, cs],
                        start=False,
                        stop=True,
                    )
                if t < 2:
                    # head tiles: argmax straight from PSUM (skips the ACT
                    # copy latency; DVE hasn't fallen behind yet so holding
                    # the PSUM slot a little longer is free)
                    s_sb = s_ps[:]
                else:
                    s_sb = s_all[:, t * NE : (t + 1) * NE]
                    nc.scalar.activation(
                        s_sb, s_ps[:], mybir.ActivationFunctionType.Copy
                    )
                max8 = m_all[:, t * 8 : (t + 1) * 8]
                idx8 = idx_all[:, t * 8 : (t + 1) * 8]
                nc.vector.max(out=max8, in_=s_sb)
                nc.vector.max_index(idx8, max8, s_sb)
                # gather this tile's codebook rows (one row per partition)
                nc.gpsimd.indirect_dma_start(
                    out=zq_sb[:, t * D : (t + 1) * D],
                    out_offset=None,
                    in_=emb_dram,
                    in_offset=bass.IndirectOffsetOnAxis(
                        ap=idx_all[:, t * 8 : t * 8 + 1], axis=0
                    ),
                )
                # flush outputs: 4-tile chunks, but the final stretch flushes
                # per tile so the kernel tail only waits on tile 31's gather
                if (t % 4 == 3 and t < 28) or t >= 28:
                    lo = t - 3 if (t % 4 == 3 and t < 28) else t
                    gs = slice(lo, t + 1)
                    nc.sync.dma_start(
                        out=zq_dram.rearrange("(g p) d -> p g d", p=P)[:, gs, :],
                        in_=zq_sb[:].rearrange("p (g d) -> p g d", g=NT)[:, gs, :],
                    )
                    es = slice(lo * 8, (t + 1) * 8)
                    nc.sync.dma_start(out=m8_dram[:, es], in_=m_all[:, es])
                    nc.sync.dma_start(out=i8_dram[:, es], in_=idx_all[:, es])

    nc.finalize()
    return nc


def _get_nc():
    key = "v2"
    if key not in _compiled:
        _compiled[key] = _build()
    return _compiled[key]


def kernel(z_real, z_imag, embedding):
    from concourse.bass_utils import run_bass_kernel_spmd

    z_real = np.ascontiguousarray(z_real, dtype=np.float32)
    embedding = np.ascontiguousarray(embedding, dtype=np.float32)

    z16 = z_real.astype(np.float16)
    embT16 = np.ascontiguousarray(embedding.T.astype(np.float16))
    ebias64 = -0.5 * (embedding.astype(np.float64) ** 2).sum(axis=1)
    ebias16 = ebias64.astype(np.float16)

    nc = _get_nc()
    in_maps = [
        {
            "z16": z16[b],
            "embT16": embT16,
            "ebias16": ebias16[None, :],
            "emb": embedding,
        }
        for b in range(B)
    ]
    try:
        res = run_bass_kernel_spmd(nc, in_maps, list(range(B)))
    except Exception:
        # transient device wedge (NRT_EXEC_UNIT_UNRECOVERABLE) heals on retry
        import time as _time

        _time.sleep(2.0)
        res = run_bass_kernel_spmd(nc, in_maps, list(range(B)))
    global LAST_RESULT
    LAST_RESULT = res

    e64 = embedding.astype(np.float64)
    eb64 = -0.5 * (e64**2).sum(axis=1)
    zq = np.empty((B, L, D), dtype=np.float32)
    zsq = (z_real.astype(np.float64) ** 2).sum()
    tot = zsq
    for b in range(B):
        r = res.results[b]
        zq[b] = r["zq"]
        # m8/i8: [P, NT, 8]; row g*128+p -> [p, g]
        m8 = r["m8out"].reshape(P, NT, 8)
        i8 = r["i8out"].reshape(P, NT, 8)
        msum = m8[:, :, 0].astype(np.float64).sum()
        # exact rescoring of near-ties among the device's top-8
        amb = np.argwhere(m8[:, :, 0] - m8[:, :, 1] < MARGIN)
        if len(amb):
            zb = z_real[b].reshape(L, D).astype(np.float64)
            pp, gg = amb[:, 0], amb[:, 1]
            rows = gg * P + pp
            cand = i8[pp, gg].astype(np.int64)  # (n, 8)
            s = np.einsum("nd,nkd->nk", zb[rows], e64[cand]) + eb64[cand]
            kbest = np.argmax(s, axis=1)
            n = np.arange(len(rows))
            best = cand[n, kbest]
            msum += (s[n, kbest] - m8[pp, gg, 0].astype(np.float64)).sum()
            zq[b, rows] = embedding[best]
        tot -= 2.0 * msum

    vq_loss = np.float32(1.25 * tot / (B * L * D))
    # straight-through estimator, replicated in fp32 exactly as the ref does
    zq_out_real = z_real + (zq - z_real)
    return zq_out_real, z_imag, vq_loss


# revision 32
# speedup vs baseline: 1.1607x; 1.1607x over previous
"""VQ codebook kernel for Trainium2 (8 NeuronCores, data-parallel over B).

Per core b: z = z_real[b] (4096, 256).
  scores t[i,j] = z_i . e_j - 0.5*||e_j||^2   (argmax_j t = argmin_j ||z_i - e_j||^2)
  computed on PE in fp16 (z and codebook host-cast to fp16; z^T loaded via the
  16-bit DMA xbar transpose straight from DRAM; bias as a rank-1 fp16 matmul).
  argmax via DVE InstMax + InstMaxIndex (top-8 values+indices kept).
  codebook gather via per-tile gpsimd indirect DMA (fp32 emb rows from DRAM).
  vq_loss identity: sum((zq-z)^2) = sum_i ||z_i||^2 - 2*sum_i max_i
    (||z||^2 exactly in fp32 via ACT Square+accum on device).
Host post-pass: rows whose top-2 fp16 scores are within MARGIN are rescored
exactly (fp64) among the device's top-8 candidates, fixing any fp16-induced
argmax flips; this reproduces full-fp32 fidelity (reference-vs-fp64 flips only
occur below fp32 noise, far inside MARGIN).
z_imag passes through on the host.
"""

import os
import sys

import numpy as np

sys.path.insert(0, "/opt/trn_rl_repo")

B, L, D, NE = 8, 4096, 256, 1024
P = 128  # partitions
NT = L // P  # 32 row tiles per core
NB = NE // 512  # 2 PSUM banks of 512 scores

_compiled = {}
LAST_RESULT = None

# rescore rows whose top-2 device scores are closer than the max plausible
# device score error (worst-case fp16 product rounding ~0.1 + fp16 bias
# rounding ~0.06, x2+ safety)
MARGIN = 0.5


def _build():
    import concourse.bass as bass
    import concourse.mybir as mybir
    from concourse import bacc
    from concourse.tile import TileContext

    f32 = mybir.dt.float32
    f16 = mybir.dt.float16

    nc = bacc.Bacc("TRN2", target_bir_lowering=False, debug=False)

    z16_dram = nc.dram_tensor("z16", [L, D], f16, kind="ExternalInput").ap()
    embT_dram = nc.dram_tensor("embT16", [D, NE], f16, kind="ExternalInput").ap()
    ebias_dram = nc.dram_tensor("ebias16", [1, NE], f16, kind="ExternalInput").ap()
    emb_dram = nc.dram_tensor("emb", [NE, D], f32, kind="ExternalInput").ap()
    zq_dram = nc.dram_tensor("zq", [L, D], f32, kind="ExternalOutput").ap()
    m8_dram = nc.dram_tensor("m8out", [P, NT * 8], f32, kind="ExternalOutput").ap()
    i8_dram = nc.dram_tensor(
        "i8out", [P, NT * 8], mybir.dt.uint32, kind="ExternalOutput"
    ).ap()

    with TileContext(nc) as tc:
        with (
            tc.tile_pool(name="persist", bufs=1) as persist,
            tc.tile_pool(name="sc", bufs=6) as sc_pool,
            tc.tile_pool(name="ps_s", bufs=4, space="PSUM") as ps_s,
        ):
            # --- codebook (transposed, fp16) + bias row + ones row FIRST so
            # the PE warmup and first tiles start as early as possible
            eT = []
            for k in range(2):
                t = persist.tile([P, NE], f16, tag=f"eT{k}")
                nc.sync.dma_start(out=t[:], in_=embT_dram[k * P : (k + 1) * P, :])
                eT.append(t)
            ebias = persist.tile([1, NE], f16, tag="ebias")
            nc.sync.dma_start(out=ebias[:], in_=ebias_dram[:])
            ones = persist.tile([1, P], f16, tag="ones")
            nc.vector.memset(ones[:], 1.0)

            # --- z^T via 16-bit xbar transpose: zt[k] [128, 4096] fp16
            # (interleave k so tile 0's both k-slices arrive first)
            zt0 = persist.tile([P, L], f16, tag="zt0")
            zt1 = persist.tile([P, L], f16, tag="zt1")
            zt = [zt0, zt1]
            for c in range(4):
                rs = slice(c * (L // 4), (c + 1) * (L // 4))
                for k in range(2):
                    nc.sync.dma_start(
                        out=zt[k][:, rs],
                        in_=z16_dram[rs, k * P : (k + 1) * P],
                        transpose=True,
                    )

            # --- accumulators
            s_all = persist.tile([P, NT * NE], f32, tag="s_all")
            m_all = persist.tile([P, NT * 8], f32, tag="m_all")
            idx_all = persist.tile([P, NT * 8], mybir.dt.uint32, tag="idx_all")
            zq_sb = persist.tile([P, NT * D], f32, tag="zq_sb")

            # --- PE warmup burst (engage HAM 2.4 GHz before the real work);
            # sourced from a memset tile so it needs no DMA and starts at t=0
            wsrc = persist.tile([P, 512], f16, tag="wsrc")
            nc.vector.memset(wsrc[:], 0.5)
            warm_ps = ps_s.tile([P, 512], f32, tag="s_ps")
            for _ in range(14):
                nc.tensor.matmul(
                    warm_ps[:], lhsT=wsrc[:, 0:P], rhs=wsrc[:],
                    start=True, stop=True,
                )

            # --- main loop over row tiles
            for t in range(NT):
                s_ps = ps_s.tile([P, NE], f32)
                # k-major order so consecutive matmuls share the stationary
                # operand (fewer weight reloads); bank accumulation groups
                # interleave via start/stop flags per PSUM region.
                for k in range(2):
                    for b in range(NB):
                        cs = slice(b * 512, (b + 1) * 512)
                        nc.tensor.matmul(
                            s_ps[:, cs],
                            lhsT=zt[k][:, t * P : (t + 1) * P],
                            rhs=eT[k][:, cs],
                            start=(k == 0),
                            stop=False,
                        )
                for b in range(NB):
                    cs = slice(b * 512, (b + 1) * 512)
                    nc.tensor.matmul(
                        s_ps[:, cs],
                        lhsT=ones[:, :],
                        rhs=ebias[:, cs],
                        start=False,
                        stop=True,
                    )
                s_sb = s_all[:, t * NE : (t + 1) * NE]
                nc.scalar.activation(
                    s_sb, s_ps[:], mybir.ActivationFunctionType.Copy
                )
                max8 = m_all[:, t * 8 : (t + 1) * 8]
                idx8 = idx_all[:, t * 8 : (t + 1) * 8]
                nc.vector.max(out=max8, in_=s_sb)
                nc.vector.max_index(idx8, max8, s_sb)
                # gather this tile's codebook rows (one row per partition)
                nc.gpsimd.indirect_dma_start(
                    out=zq_sb[:, t * D : (t + 1) * D],
                    out_offset=None,
                    in_=emb_dram,
                    in_offset=bass.IndirectOffsetOnAxis(
                        ap=idx_all[:, t * 8 : t * 8 + 1], axis=0
                    ),
                )
                # flush outputs: 4-tile chunks, but the final stretch flushes
                # per tile so the kernel tail only waits on tile 31's gather
                if (t % 4 == 3 and t < 28) or t >= 28:
                    lo = t - 3 if (t % 4 == 3 and t < 28) else t
                    gs = slice(lo, t + 1)
                    nc.sync.dma_start(
                        out=zq_dram.rearrange("(g p) d -> p g d", p=P)[:, gs, :],
                        in_=zq_sb[:].rearrange("p (g d) -> p g d", g=NT)[:, gs, :],
                    )
                    es = slice(lo * 8, (t + 1) * 8)
                    nc.sync.dma_start(out=m8_dram[:, es], in_=m_all[:, es])
                    nc.sync.dma_start(out=i8_dram[:, es], in_=idx_all[:, es])

    nc.finalize()
    return nc


def _get_nc():
    key = "v2"
    if key not in _compiled:
        _compiled[key] = _build()
    return _compiled[key]


def kernel(z_real, z_imag, embedding):
    from concourse.bass_utils import run_bass_kernel_spmd

    z_real = np.ascontiguousarray(z_real, dtype=np.float32)
    embedding = np.ascontiguousarray(embedding, dtype=np.float32)

    z16 = z_real.astype(np.float16)
    embT16 = np.ascontiguousarray(embedding.T.astype(np.float16))
    ebias64 = -0.5 * (embedding.astype(np.float64) ** 2).sum(axis=1)
    ebias16 = ebias64.astype(np.float16)

    nc = _get_nc()
    in_maps = [
        {
            "z16": z16[b],
            "embT16": embT16,
            "ebias16": ebias16[None, :],
            "emb": embedding,
        }
        for b in range(B)
    ]
    try:
        res = run_bass_kernel_spmd(nc, in_maps, list(range(B)))
    except Exception:
        # transient device wedge (NRT_EXEC_UNIT_UNRECOVERABLE) heals on retry
        import time as _time

        _time.sleep(2.0)
        res = run_bass_kernel_spmd(nc, in_maps, list(range(B)))
    global LAST_RESULT
    LAST_RESULT = res

    e64 = embedding.astype(np.float64)
    eb64 = -0.5 * (e64**2).sum(axis=1)
    zq = np.empty((B, L, D), dtype=np.float32)
    zsq = (z_real.astype(np.float64) ** 2).sum()
    tot = zsq
    for b in range(B):
        r = res.results[b]
        zq[b] = r["zq"]
        # m8/i8: [P, NT, 8]; row g*128+p -> [p, g]
        m8 = r["m8out"].reshape(P, NT, 8)
        i8 = r["i8out"].reshape(P, NT, 8)
        msum = m8[:, :, 0].astype(np.float64).sum()
        # exact rescoring of near-ties among the device's top-8
        amb = np.argwhere(m8[:, :, 0] - m8[:, :, 1] < MARGIN)
        if len(amb):
            zb = z_real[b].reshape(L, D).astype(np.float64)
            pp, gg = amb[:, 0], amb[:, 1]
            rows = gg * P + pp
            cand = i8[pp, gg].astype(np.int64)  # (n, 8)
            s = np.einsum("nd,nkd->nk", zb[rows], e64[cand]) + eb64[cand]
            kbest = np.argmax(s, axis=1)
            n = np.arange(len(rows))
            best = cand[n, kbest]
            msum += (s[n, kbest] - m8[pp, gg, 0].astype(np.float64)).sum()
            zq[b, rows] = embedding[best]
        tot -= 2.0 * msum

    vq_loss = np.float32(1.25 * tot / (B * L * D))
    # straight-through estimator, replicated in fp32 exactly as the ref does
    zq_out_real = z_real + (zq - z_real)
    return zq_out_real, z_imag, vq_loss
